# revision 68
# baseline (speedup 1.0000x reference)
"""Trainium2 Bass kernel for nn_DiffusionActionHead (B=8, S=2048, D=4096).

Strategy (8 NeuronCores), v2:
  - MAP-head probe projection is folded on HOST: U = wk_h^T (probe wq_h + bq_h)
    / sqrt(DH) is data-independent, so scores = llm @ U directly. This removes
    the wq/wk streams, the on-device q/U matmuls and the U AllGather entirely.
  - Data-parallel over batch for scores/softmax/pooled (each core owns one
    batch row of llm); AllToAll converts pooled to head-parallel for the
    wv/wo stage (each core reads only its head slice); AllReduce after wo and
    after the TP-sharded MLP (w1 col-shard, w2 row-shard). Tail replicated.
  - llmT is streamed in fp8 e3m4 (scores are softmax-shift tolerant; host
    scales llm by 2 into e3m4 range and stores U/2). llm natural stays f16
    for the pooled pass; all weights stay f16 (fp8 on weights was measured
    to breach the 2e-2 gate).
  - All collective payloads are f16. Collective staging writes + readbacks
    ride the gpsimd SWDGE queue so a semaphore-waiting DMA never head-of-line
    blocks a weight prefetch on the HWDGE rings. A tiny AllReduce barrier at
    t~0 absorbs the kernel-launch skew; the pooled halves are AllToAll'd
    separately so the first A2A hides under the second half's compute, and
    the attn AllReduce is likewise split per half.
  - All big streams use 1-MiB DMAs alternating between the two HWDGE rings
    (0.5-MiB DMAs measured only ~220 GB/s vs ~350-400 at 1 MiB) and share
    ONE 13-buffer tile pool, so weight prefetch backlog keeps the DMA
    saturated through softmax/collective gaps and the mlp runs at the
    HBM roofline. The mlp AllReduce runs on pre-transposed [P, DC*B] data
    so the post-AR chain is just 32 matmuls into the x0 bank.
  - x0 = cond@rin_c + na@rin_na + rin_b accumulates into its PSUM bank at
    t~0 (start=True); the xp@rin_pool chunks land into the same bank after
    the last AllReduce.
  - All transposes are PE transposes with f16 in/out (PSUM f16 transpose
    path); LN affine and softmax denominators fold into the evict copies.
"""

import numpy as np
import sys

if "/opt/trn_rl_repo" not in sys.path:
    sys.path.insert(0, "/opt/trn_rl_repo")

import ml_dtypes
import concourse.bass as bass
import concourse.tile as tile
from concourse import bacc, mybir
from concourse.masks import make_identity
from concourse.bass_utils import run_bass_kernel_spmd

F32 = mybir.dt.float32
F16 = mybir.dt.float16
F8 = mybir.dt.float8e3
AF = mybir.ActivationFunctionType
ALU = mybir.AluOpType

B, S, D = 8, 2048, 4096
H, AD, TD, HID, NBLK = 8, 7, 32, 256, 3
DH = D // H            # 512
NC = 8                 # cores
P = 128
SC = S // P            # 16 S-chunks
DC = D // P            # 32 D-chunks
HD2 = D // 2           # 2048
F1S = 4 * D // NC      # 2048 per-core hidden cols of mlp_w1
HC = HID // P          # 2
LLM_SCALE = 2.0        # llm * 2 fits e3m4 comfortably; U stored as U/2
TWO_PI = 2.0 * float(np.pi)


def _bcast(src_ap, nparts):
    """Partition-broadcast a (1, N) DRAM AP to (nparts, N)."""
    ap = src_ap
    assert ap.shape[0] == 1, ap.shape
    return bass.AP(tensor=ap.tensor, offset=ap.offset,
                   ap=[[0, nparts]] + [list(x) for x in ap.ap[1:]])


def build_program():
    nc = bacc.Bacc("TRN2", target_bir_lowering=False, debug=False,
                   num_devices=NC)
    t = {}

    def din(name, shape, dtype=F32):
        t[name] = nc.dram_tensor(name, shape, dtype, kind="ExternalInput")

    din("llmT8", [D, S], F8)
    din("llm16", [S, D], F16)
    din("U16r", [P, DC, H], F16)
    din("wv_s", [D, DH], F16); din("bv16", [1, DH], F16)
    din("wo_s", [DH, D], F16); din("bo16", [1, D], F16)        # bo/8
    din("ln_g_r", [P, DC])
    din("w1_s", [D, F1S], F16)
    din("r1c", [1, F1S], F16); din("br2", [1, F1S], F16)
    din("w2_s", [F1S, D], F16); din("b216", [1, D], F16)       # b2/8
    din("four_w2", [TD, 1]); din("phase2", [TD, 1])
    din("timeT", [1, B]); din("naT", [AD, B], F16)
    din("cond_w1", [TD, 2 * TD], F16); din("cond_b1c", [2 * TD, 1])
    din("cond_w2", [2 * TD, TD], F16); din("cond_b2c", [TD, 1])
    din("rin_cond", [TD, HID], F16); din("rinp", [P, DC, HID], F16)
    din("rin_na", [AD, HID], F16); din("rb16", [1, HID], F16)
    din("blk_g_r", [NBLK, P, HC]); din("blk_b_r", [NBLK, P, HC])
    din("blkw1p", [P, NBLK * HC, 4 * HID], F16)
    din("b1r", [1, NBLK * 8, P], F16)
    din("blkw2p", [P, NBLK, 8 * HID], F16)
    din("blk_b2_16", [NBLK, HID], F16)
    din("out_w", [P, HC, AD], F16); din("out_bc", [1, AD])
    t["res"] = nc.dram_tensor("res", [B, AD], F32, kind="ExternalOutput")

    # collective bounce buffers (f16 payloads)
    t["cc_bar_in"] = nc.dram_tensor("cc_bar_in", [1, 8], F16)
    t["cc_bar_out"] = nc.dram_tensor("cc_bar_out", [1, 8], F16,
                                     addr_space="Shared")
    for hf in range(2):
        t[f"cc_pool_in{hf}"] = nc.dram_tensor(f"cc_pool_in{hf}", [H, HD2], F16)
        t[f"cc_pool_out{hf}"] = nc.dram_tensor(f"cc_pool_out{hf}", [B, HD2],
                                               F16)
        t[f"cc_attn_in{hf}"] = nc.dram_tensor(f"cc_attn_in{hf}", [B, HD2], F16)
        t[f"cc_attn_out{hf}"] = nc.dram_tensor(f"cc_attn_out{hf}", [B, HD2],
                                               F16, addr_space="Shared")
    t["cc_mlp_in"] = nc.dram_tensor("cc_mlp_in", [P, DC * B], F16)
    t["cc_mlp_out"] = nc.dram_tensor("cc_mlp_out", [P, DC * B], F16,
                                     addr_space="Shared")

    with tile.TileContext(nc) as tc:
        import contextlib
        with contextlib.ExitStack() as ctx:
            _build(nc, tc, t, ctx)
    nc.finalize()
    return nc


def _build(nc, tc, t, ctx):
    GROUPS = [list(range(NC))]

    singles = ctx.enter_context(tc.tile_pool(name="singles", bufs=1))
    stream = ctx.enter_context(tc.tile_pool(name="stream", bufs=13))
    natD = ctx.enter_context(tc.tile_pool(name="natD", bufs=2))
    nat8 = ctx.enter_context(tc.tile_pool(name="nat8", bufs=2))
    psA = ctx.enter_context(tc.tile_pool(name="psA", bufs=1, space="PSUM"))
    psB = ctx.enter_context(tc.tile_pool(name="psB", bufs=2, space="PSUM"))
    psC = ctx.enter_context(tc.tile_pool(name="psC", bufs=1, space="PSUM"))

    ident16 = singles.tile([P, P], F16)
    make_identity(nc, ident16)
    eps_sb = singles.tile([P, 1], F32)
    nc.vector.memset(eps_sb[:], 1e-5)
    ones8 = singles.tile([1, 8], F16)
    nc.vector.memset(ones8[:], 1.0)

    def evict(dst, src):
        nc.vector.tensor_copy(out=dst, in_=src)

    def t_T16(src, dst, nchunks, npart, uid, g=None, b=None):
        """(npart, nchunks*128) f16 sbuf -> (128, nchunks, npart) f16 sbuf
        via PE f16 transposes; optional per-chunk affine (g, b are (P, nchunks)
        f32 tiles applied per-partition on the transposed data)."""
        for c in range(nchunks):
            ps = psB.tile([P, 8], F16, tag="tp16", name=f"tp_{uid}_{c}")
            nc.tensor.transpose(ps[:, :npart], src[:, c * P:(c + 1) * P],
                                ident16[:npart, :npart])
            if g is not None:
                nc.vector.tensor_scalar(out=dst[:, c, :], in0=ps[:, :npart],
                                        scalar1=g[:, c:c + 1],
                                        scalar2=b[:, c:c + 1],
                                        op0=ALU.mult, op1=ALU.add)
            else:
                nc.vector.tensor_copy(out=dst[:, c, :], in_=ps[:, :npart])

    def bias_mm(ps, bias_row, n_total, stop=True):
        """Add a (1, n_total) f16 bias row into psum (8, n_total) via ones-row
        matmuls, 512 cols per matmul."""
        nch = (n_total + 511) // 512
        for n in range(nch):
            w = min(512, n_total - n * 512)
            nc.tensor.matmul(ps[:, n * 512:n * 512 + w], ones8[:, :B],
                             bias_row[:, n * 512:n * 512 + w],
                             start=False, stop=(stop and n == nch - 1))

    def layernorm16(x_in, npart, n, y16, uid):
        """y16 = f16((x - mean) / sqrt(var + eps)) over free dim of (npart, n).
        Input may be f16 or f32."""
        nsub = max(1, n // 512)
        st = nat8.tile([npart, nsub, nc.vector.BN_STATS_DIM], F32, tag="lnst",
                       name=f"lnst_{uid}")
        xg = x_in.rearrange("p (a b) -> p a b", a=nsub)
        for g in range(nsub):
            nc.vector.bn_stats(out=st[:, g, :], in_=xg[:, g, :])
        mv = nat8.tile([npart, nc.vector.BN_AGGR_DIM], F32, tag="lnmv",
                       name=f"lnmv_{uid}")
        nc.vector.bn_aggr(out=mv[:], in_=st[:])
        std = nat8.tile([npart, 1], F32, tag="lnsd", name=f"lnsd_{uid}")
        nc.scalar.activation(out=std[:], in_=mv[:, 1:2], func=AF.Sqrt,
                             bias=eps_sb[:npart, :])
        nc.vector.reciprocal(out=std[:], in_=std[:])
        nc.vector.tensor_scalar(out=y16, in0=x_in, scalar1=mv[:, 0:1],
                                scalar2=std[:], op0=ALU.subtract, op1=ALU.mult)

    # =======================================================================
    # STEP 0: small constants on gpsimd/scalar queues (prefetched early).
    # =======================================================================
    U16 = singles.tile([P, DC, H], F16)
    nc.scalar.dma_start(out=U16[:], in_=t["U16r"][:])
    cw1_sb = singles.tile([TD, 2 * TD], F16)
    nc.scalar.dma_start(out=cw1_sb[:], in_=t["cond_w1"][:])
    cw2_sb = singles.tile([2 * TD, TD], F16)
    nc.scalar.dma_start(out=cw2_sb[:], in_=t["cond_w2"][:])
    rc_sb = singles.tile([TD, HID], F16)
    nc.scalar.dma_start(out=rc_sb[:], in_=t["rin_cond"][:])
    rna_sb = singles.tile([AD, HID], F16)
    nc.scalar.dma_start(out=rna_sb[:], in_=t["rin_na"][:])

    bv_sb = singles.tile([1, DH], F16)
    nc.gpsimd.dma_start(out=bv_sb[:], in_=t["bv16"][:])
    bo_sb = singles.tile([1, D], F16)
    nc.gpsimd.dma_start(out=bo_sb[:], in_=t["bo16"][:])
    r1_sb = singles.tile([1, F1S], F16)
    nc.gpsimd.dma_start(out=r1_sb[:], in_=t["r1c"][:])
    br2_sb = singles.tile([1, F1S], F16)
    nc.gpsimd.dma_start(out=br2_sb[:], in_=t["br2"][:])
    b2_sb = singles.tile([1, D], F16)
    nc.gpsimd.dma_start(out=b2_sb[:], in_=t["b216"][:])
    rb_sb = singles.tile([1, HID], F16)
    nc.gpsimd.dma_start(out=rb_sb[:], in_=t["rb16"][:])
    b1r_sb = singles.tile([1, NBLK * 8, P], F16)
    nc.gpsimd.dma_start(out=b1r_sb[:], in_=t["b1r"][:])
    bb2_sb = singles.tile([1, NBLK, HID], F16)
    nc.gpsimd.dma_start(out=bb2_sb[:],
                        in_=t["blk_b2_16"][:].rearrange("n f -> (n f)")[None, :])
    lng_sb = singles.tile([P, DC], F32)
    nc.gpsimd.dma_start(out=lng_sb[:], in_=t["ln_g_r"][:])

    bgr_sb = singles.tile([P, NBLK, HC], F32)
    nc.gpsimd.dma_start(out=bgr_sb[:],
                        in_=t["blk_g_r"][:].rearrange("n p c -> p n c"))
    bbr_sb = singles.tile([P, NBLK, HC], F32)
    nc.gpsimd.dma_start(out=bbr_sb[:],
                        in_=t["blk_b_r"][:].rearrange("n p c -> p n c"))
    naT_sb = singles.tile([AD, B], F16)
    nc.gpsimd.dma_start(out=naT_sb[:], in_=t["naT"][:])
    ow_sb = singles.tile([P, HC, AD], F16)
    nc.gpsimd.dma_start(out=ow_sb[:], in_=t["out_w"][:])
    ob_bc = singles.tile([B, AD], F32)
    nc.gpsimd.dma_start(out=ob_bc[:], in_=_bcast(t["out_bc"][:], B))
    fw_sb = singles.tile([TD, 1], F32)
    nc.gpsimd.dma_start(out=fw_sb[:], in_=t["four_w2"][:])
    ph_sb = singles.tile([TD, 1], F32)
    nc.gpsimd.dma_start(out=ph_sb[:], in_=t["phase2"][:])
    cb1_sb = singles.tile([2 * TD, 1], F32)
    nc.gpsimd.dma_start(out=cb1_sb[:], in_=t["cond_b1c"][:])
    cb2_sb = singles.tile([TD, 1], F32)
    nc.gpsimd.dma_start(out=cb2_sb[:], in_=t["cond_b2c"][:])
    tb32 = singles.tile([TD, B], F32)
    nc.gpsimd.dma_start(out=tb32[:], in_=_bcast(t["timeT"][:], TD))

    # launch-skew barrier: a tiny AllReduce so cores re-sync long before the
    # first real collective (its wait overlaps the scores/pooled phases).
    nc.gpsimd.dma_start(out=t["cc_bar_in"][:], in_=ones8[:])
    nc.gpsimd.collective_compute(
        "AllReduce", ALU.add, replica_groups=GROUPS,
        ins=[t["cc_bar_in"][:].opt()], outs=[t["cc_bar_out"][:].opt()])

    # =======================================================================
    # STEP 1: cond path (fourier + tiny mlp) — independent of everything,
    # done first so condT exists before the early x0 accumulation.
    # =======================================================================
    fu = singles.tile([TD, B], F32)
    nc.vector.tensor_scalar_mul(out=fu[:], in0=tb32[:], scalar1=fw_sb[:])
    fi = singles.tile([TD, B], mybir.dt.int32)
    nc.vector.tensor_copy(out=fi[:], in_=fu[:])
    fif = singles.tile([TD, B], F32)
    nc.vector.tensor_copy(out=fif[:], in_=fi[:])
    nc.vector.tensor_sub(out=fu[:], in0=fu[:], in1=fif[:])
    ffT = singles.tile([TD, B], F16)
    nc.scalar.activation(out=ffT[:], in_=fu[:], func=AF.Sin,
                         scale=TWO_PI, bias=ph_sb[:])
    ps_c1 = psC.tile([2 * TD, B], F32, tag="mix", name="ps_c1")
    nc.tensor.matmul(ps_c1[:], cw1_sb[:], ffT[:], start=True, stop=True)
    c1 = singles.tile([2 * TD, B], F16)
    nc.scalar.activation(out=c1[:], in_=ps_c1[:], func=AF.Silu,
                         bias=cb1_sb[:])
    ps_c2 = psC.tile([TD, B], F32, tag="mix", name="ps_c2")
    nc.tensor.matmul(ps_c2[:], cw2_sb[:], c1[:], start=True, stop=True)
    condT = singles.tile([TD, B], F16)
    nc.scalar.activation(out=condT[:], in_=ps_c2[:], func=AF.Identity,
                         bias=cb2_sb[:])

    # x0 accumulation bank: cond + noisy_actions + bias land now, the
    # xp @ rin_pool chunks land after the mlp AllReduce.
    ps_x0 = psC.tile([B, HID], F32, tag="x0", name="ps_x0")
    nc.tensor.matmul(ps_x0[:], condT[:], rc_sb[:], start=True, stop=False)
    nc.tensor.matmul(ps_x0[:], naT_sb[:], rna_sb[:], start=False, stop=False)
    bias_mm(ps_x0, rb_sb, HID, stop=False)

    # =======================================================================
    # STEP 2: scoresT (8, 2048) = U.T @ llmT8  (f16 x fp8, fp32 accum)
    # =======================================================================
    def dq(i):
        return nc.sync if i % 2 == 0 else nc.scalar

    ps_sc = psA.tile([H, S], F32, tag="big", name="ps_sc")
    for kt in range(DC // 4):
        lt = stream.tile([P, 4, S], F8, tag="st", name=f"l8_{kt}")
        dq(kt).dma_start(
            out=lt[:],
            in_=t["llmT8"][kt * 512:(kt + 1) * 512, :].rearrange(
                "(a p) s -> p a s", a=4))
        for a in range(4):
            k = 4 * kt + a
            for n in range(S // 512):
                nc.tensor.matmul(ps_sc[:, n * 512:(n + 1) * 512],
                                 U16[:, k, :], lt[:, a, n * 512:(n + 1) * 512],
                                 start=(k == 0), stop=(k == DC - 1))

    # =======================================================================
    # STEP 3: softmax (shift-free: |scores| < ~0.2). The denominator is
    # folded into the pooled evict.
    # =======================================================================
    p16 = natD.tile([H, S], F16, tag="nat", name="p16")
    nc.scalar.activation(out=p16[:], in_=ps_sc[:], func=AF.Exp)
    den = singles.tile([H, 1], F32)
    nc.vector.reduce_sum(out=den[:], in_=p16[:], axis=mybir.AxisListType.X)
    nc.vector.reciprocal(out=den[:], in_=den[:])
    pT = singles.tile([P, SC, H], F16)  # chunks built inside the half-0 loop

    # =======================================================================
    # STEP 4+5: pooled halves, each AllToAll'd as soon as it is ready; the
    # half-0 A2A + readback + transposes + ctx chunks hide under half 1.
    # =======================================================================
    pool16 = natD.tile([H, D], F16, tag="nat", name="pool16")
    poolh16 = natD.tile([B, D], F16, tag="nat", name="poolh16")
    poolhT = singles.tile([P, DC, B], F16)
    ps_cx = psC.tile([B, DH], F32, tag="mix", name="ps_cx")
    wv_r = t["wv_s"].rearrange("(c p) n -> p c n", p=P)
    wv_ts = []

    # half 0 pooled
    ps_p0 = psA.tile([H, HD2], F32, tag="big", name="ps_pool_0")
    for s2 in range(SC // 2):
        lt = stream.tile([P, 2, HD2], F16, tag="st", name=f"llm_0_{s2}")
        dq(s2).dma_start(
            out=lt[:],
            in_=t["llm16"][s2 * 256:(s2 + 1) * 256, :HD2].rearrange(
                "(a p) n -> p a n", a=2))
        for a in range(2):
            s = 2 * s2 + a
            ps = psB.tile([P, 8], F16, tag="tp16", name=f"tp_p_{s}")
            nc.tensor.transpose(ps[:, :H], p16[:, s * P:(s + 1) * P],
                                ident16[:H, :H])
            nc.vector.tensor_copy(out=pT[:, s, :], in_=ps[:, :H])
            for n in range(HD2 // 512):
                nc.tensor.matmul(ps_p0[:, n * 512:(n + 1) * 512],
                                 pT[:, s, :], lt[:, a, n * 512:(n + 1) * 512],
                                 start=(s == 0), stop=(s == SC - 1))
    nc.vector.tensor_scalar_mul(out=pool16[:, :HD2], in0=ps_p0[:],
                                scalar1=den[:])
    nc.gpsimd.dma_start(out=t["cc_pool_in0"][:], in_=pool16[:, :HD2])
    nc.gpsimd.collective_compute(
        "AllToAll", ALU.bypass, replica_groups=GROUPS,
        ins=[t["cc_pool_in0"][:].opt()], outs=[t["cc_pool_out0"][:].opt()])
    nc.gpsimd.dma_start(out=poolh16[:, :HD2], in_=t["cc_pool_out0"][:])

    # half 1 pooled
    ps_p1 = psA.tile([H, HD2], F32, tag="big", name="ps_pool_1")
    for s2 in range(SC // 2):
        lt = stream.tile([P, 2, HD2], F16, tag="st", name=f"llm_1_{s2}")
        dq(s2 + 1).dma_start(
            out=lt[:],
            in_=t["llm16"][s2 * 256:(s2 + 1) * 256, HD2:].rearrange(
                "(a p) n -> p a n", a=2))
        for a in range(2):
            s = 2 * s2 + a
            for n in range(HD2 // 512):
                nc.tensor.matmul(ps_p1[:, n * 512:(n + 1) * 512],
                                 pT[:, s, :], lt[:, a, n * 512:(n + 1) * 512],
                                 start=(s == 0), stop=(s == SC - 1))
    nc.vector.tensor_scalar_mul(out=pool16[:, HD2:], in0=ps_p1[:],
                                scalar1=den[:])
    nc.gpsimd.dma_start(out=t["cc_pool_in1"][:], in_=pool16[:, HD2:])
    nc.gpsimd.collective_compute(
        "AllToAll", ALU.bypass, replica_groups=GROUPS,
        ins=[t["cc_pool_in1"][:].opt()], outs=[t["cc_pool_out1"][:].opt()])
    nc.gpsimd.dma_start(out=poolh16[:, HD2:], in_=t["cc_pool_out1"][:])

    # wv tiles queued AFTER the llm stream so they never delay the pooled
    # critical path (ctx consumes them well after the half-1 A2A).
    for g in range(4):
        wt = stream.tile([P, 8, DH], F16, tag="st", name=f"wv_g{g}")
        dq(g).dma_start(out=wt[:], in_=wv_r[:, 8 * g:8 * g + 8, :])
        wv_ts.append(wt)

    # ctx accumulation: chunks 0-15 depend only on the half-0 A2A (long done);
    # chunks 16-31 wait on the half-1 A2A, hidden under the first 16.
    def ctx_chunk(k):
        ps = psB.tile([P, 8], F16, tag="tp16", name=f"tp_ph_{k}")
        nc.tensor.transpose(ps[:, :B], poolh16[:, k * P:(k + 1) * P],
                            ident16[:B, :B])
        nc.vector.tensor_copy(out=poolhT[:, k, :], in_=ps[:, :B])
        nc.tensor.matmul(ps_cx[:], poolhT[:, k, :], wv_ts[k // 8][:, k % 8, :],
                         start=(k == 0), stop=False)

    for k in range(DC):
        ctx_chunk(k)
    bias_mm(ps_cx, bv_sb, DH)
    ctx16 = singles.tile([B, DH], F16)
    evict(ctx16[:], ps_cx[:])
    ctxT = singles.tile([P, DH // P, B], F16)  # built inside attn half 0

    # =======================================================================
    # STEP 6: attn partial halves = ctx @ wo_s + bo/8 ; AllReduce f16 per
    # half, half-0's AR + readback + bn_stats hidden under half 1.
    # =======================================================================
    attn16 = natD.tile([B, D], F16, tag="nat", name="attn16")
    attn16p = singles.tile([B, D], F16)  # persists (residual)
    st_y = nat8.tile([B, 8, nc.vector.BN_STATS_DIM], F32, tag="lnst",
                     name="st_y")
    for half in range(2):
        ps_a = psA.tile([B, HD2], F32, tag="big", name=f"ps_attn_{half}")
        for kt in range(2):
            wt = stream.tile([P, 2, HD2], F16, tag="st",
                               name=f"wo_t{half}_{kt}")
            dq(kt + half).dma_start(
                out=wt[:],
                in_=t["wo_s"][kt * 256:(kt + 1) * 256,
                              half * HD2:(half + 1) * HD2].rearrange(
                    "(a p) n -> p a n", a=2))
            for a in range(2):
                k = 2 * kt + a
                if half == 0:
                    ps = psB.tile([P, 8], F16, tag="tp16", name=f"tp_cx_{k}")
                    nc.tensor.transpose(ps[:, :B], ctx16[:, k * P:(k + 1) * P],
                                        ident16[:B, :B])
                    nc.vector.tensor_copy(out=ctxT[:, k, :], in_=ps[:, :B])
                for n in range(HD2 // 512):
                    nc.tensor.matmul(ps_a[:, n * 512:(n + 1) * 512],
                                     ctxT[:, k, :],
                                     wt[:, a, n * 512:(n + 1) * 512],
                                     start=(k == 0), stop=False)
        bias_mm(ps_a, bo_sb[:, half * HD2:(half + 1) * HD2], HD2)
        evict(attn16[:, half * HD2:(half + 1) * HD2], ps_a[:])
        nc.gpsimd.dma_start(out=t[f"cc_attn_in{half}"][:],
                            in_=attn16[:, half * HD2:(half + 1) * HD2])
        nc.gpsimd.collective_compute(
            "AllReduce", ALU.add, replica_groups=GROUPS,
            ins=[t[f"cc_attn_in{half}"][:].opt()],
            outs=[t[f"cc_attn_out{half}"][:].opt()])
        nc.gpsimd.dma_start(out=attn16p[:, half * HD2:(half + 1) * HD2],
                            in_=t[f"cc_attn_out{half}"][:])
        xg = attn16p[:, half * HD2:(half + 1) * HD2].rearrange(
            "p (a b) -> p a b", a=4)
        for g in range(4):
            nc.vector.bn_stats(out=st_y[:, half * 4 + g, :], in_=xg[:, g, :])

    # =======================================================================
    # STEP 7: mlp with the LayerNorm folded into mm1:
    #   h1 = rstd*[(attn*g)@w1] - m*rstd*(g@w1) + (lnb@w1 + b1)
    #      = rstd * [ T1 + (-m)*r1 + (1/rstd)*br2 ]
    # where r1 = g@w1 and br2 = lnb@w1 + b1 are host-precomputed rows. T1's
    # accumulation starts right after each AR1-half readback (no stats dep),
    # so AR1_1 + the LN chain hide under mm1's first half.
    # =======================================================================
    attngT = singles.tile([P, DC, B], F16)  # (attn * g).T chunks

    def attng_chunk(c):
        ps = psB.tile([P, 8], F16, tag="tp16", name=f"tp_ag_{c}")
        nc.tensor.transpose(ps[:, :B], attn16p[:, c * P:(c + 1) * P],
                            ident16[:B, :B])
        nc.vector.tensor_scalar_mul(out=attngT[:, c, :], in0=ps[:, :B],
                                    scalar1=lng_sb[:, c:c + 1])

    for c in range(DC // 2):
        attng_chunk(c)

    # rin_pool chunks ride the stream pool ahead of w1 (used during mm1 by
    # the x0 attn-part matmuls interleaved below).
    rp_sb = []
    for j in range(DC // 16):
        wt = stream.tile([P, 16, HID], F16, tag="st", name=f"rp_{j}")
        dq(j).dma_start(out=wt[:], in_=t["rinp"][:, 16 * j:16 * j + 16, :])
        rp_sb.append(wt)

    attnT = singles.tile([P, DC, B], F16)
    ps_h1 = psA.tile([B, F1S], F32, tag="big", name="ps_h1")
    for kt in range(DC // 2):
        wt = stream.tile([P, 2, F1S], F16, tag="st", name=f"w1_t{kt}")
        dq(kt).dma_start(
            out=wt[:],
            in_=t["w1_s"][kt * 256:(kt + 1) * 256, :].rearrange(
                "(a p) n -> p a n", a=2))
        for a in range(2):
            k = 2 * kt + a
            if k >= DC // 2:
                attng_chunk(k)
            for n in range(F1S // 512):
                nc.tensor.matmul(ps_h1[:, n * 512:(n + 1) * 512],
                                 attngT[:, k, :],
                                 wt[:, a, n * 512:(n + 1) * 512],
                                 start=(k == 0), stop=False)
        for a in range(2):
            k = 2 * kt + a
            ps = psB.tile([P, 8], F16, tag="tp16", name=f"tp_at_{k}")
            nc.tensor.transpose(ps[:, :B], attn16p[:, k * P:(k + 1) * P],
                                ident16[:B, :B])
            nc.vector.tensor_copy(out=attnT[:, k, :], in_=ps[:, :B])
            nc.tensor.matmul(ps_x0[:], attnT[:, k, :],
                             rp_sb[k // 16][:, k % 16, :],
                             start=False, stop=False)

    # LN stats -> (-m | std) row for the two rank-1 correction matmuls
    mv_y = nat8.tile([B, nc.vector.BN_AGGR_DIM], F32, tag="lnmv", name="mv_y")
    nc.vector.bn_aggr(out=mv_y[:], in_=st_y[:])
    stdv = nat8.tile([B, 1], F32, tag="lnsd", name="stdv")
    nc.scalar.activation(out=stdv[:], in_=mv_y[:, 1:2], func=AF.Sqrt,
                         bias=eps_sb[:B, :])
    rstd = nat8.tile([B, 1], F32, tag="lnsd", name="rstd")
    nc.vector.reciprocal(out=rstd[:], in_=stdv[:])
    mscol = singles.tile([B, 2], F16)
    nc.vector.tensor_scalar_mul(out=mscol[:, 0:1], in0=mv_y[:, 0:1],
                                scalar1=-1.0)
    nc.vector.tensor_copy(out=mscol[:, 1:2], in_=stdv[:])
    ps_m = psB.tile([P, 8], F16, tag="tp16", name="tp_msM")
    nc.tensor.transpose(ps_m[:1, :B], mscol[:, 0:1], ident16[:B, :B])
    rowM = singles.tile([1, B], F16)
    evict(rowM[:], ps_m[:1, :B])
    ps_s = psB.tile([P, 8], F16, tag="tp16", name="tp_msS")
    nc.tensor.transpose(ps_s[:1, :B], mscol[:, 1:2], ident16[:B, :B])
    rowS = singles.tile([1, B], F16)
    evict(rowS[:], ps_s[:1, :B])
    for n in range(F1S // 512):
        nc.tensor.matmul(ps_h1[:, n * 512:(n + 1) * 512], rowM[:],
                         r1_sb[:, n * 512:(n + 1) * 512],
                         start=False, stop=False)
    for n in range(F1S // 512):
        nc.tensor.matmul(ps_h1[:, n * 512:(n + 1) * 512], rowS[:],
                         br2_sb[:, n * 512:(n + 1) * 512],
                         start=False, stop=True)
    g16 = natD.tile([B, F1S], F16, tag="nat", name="g16")
    nc.scalar.activation(out=g16[:], in_=ps_h1[:], func=AF.Gelu,
                         scale=rstd[:])
    gT = singles.tile([P, F1S // P, B], F16)  # built inside mm2 half 0

    bw1_sb = []
    for i in range(NBLK):
        for k in range(HC):
            wt = singles.tile([P, 4 * HID], F16, name=f"bw1_{i}_{k}")
            nc.scalar.dma_start(out=wt[:], in_=t["blkw1p"][:, i * HC + k, :])
            bw1_sb.append(wt)
    bw2_sb = []
    for i in range(NBLK):
        wt = singles.tile([P, 8, HID], F16, name=f"bw2_{i}")
        nc.scalar.dma_start(out=wt[:], in_=t["blkw2p"][:, i, :].rearrange(
            "p (a n) -> p a n", a=8))
        bw2_sb.append(wt)

    # mm2: h2 partial (8, 4096) = g @ w2_s + b2/8. The partial is transposed
    # on the fly (interleaved with the DMA-paced w2 stream) and AllReduced in
    # T layout [P, DC*B] so the post-AR chain is just 32 matmuls.
    h216 = natD.tile([B, D], F16, tag="nat", name="h216")
    h2T = singles.tile([P, DC, B], F16)
    for half in range(2):
        ps_h2 = psA.tile([B, HD2], F32, tag="big", name=f"ps_h2_{half}")
        for kt in range(F1S // 256):
            wt = stream.tile([P, 2, HD2], F16, tag="st",
                              name=f"w2_t{half}_{kt}")
            dq(kt + half).dma_start(
                out=wt[:],
                in_=t["w2_s"][kt * 256:(kt + 1) * 256,
                              half * HD2:(half + 1) * HD2].rearrange(
                    "(a p) n -> p a n", a=2))
            for a in range(2):
                k = 2 * kt + a
                if half == 0:
                    ps = psB.tile([P, 8], F16, tag="tp16", name=f"tp_g_{k}")
                    nc.tensor.transpose(ps[:, :B], g16[:, k * P:(k + 1) * P],
                                        ident16[:B, :B])
                    nc.vector.tensor_copy(out=gT[:, k, :], in_=ps[:, :B])
                for n in range(HD2 // 512):
                    nc.tensor.matmul(ps_h2[:, n * 512:(n + 1) * 512],
                                     gT[:, k, :],
                                     wt[:, a, n * 512:(n + 1) * 512],
                                     start=(k == 0), stop=False)
            if half == 1:
                for a in range(2):
                    c = 2 * kt + a  # transpose half0's chunks under half1
                    ps = psB.tile([P, 8], F16, tag="tp16", name=f"tp_h2_{c}")
                    nc.tensor.transpose(ps[:, :B], h216[:, c * P:(c + 1) * P],
                                        ident16[:B, :B])
                    nc.vector.tensor_copy(out=h2T[:, c, :], in_=ps[:, :B])
        bias_mm(ps_h2, b2_sb[:, half * HD2:(half + 1) * HD2], HD2)
        evict(h216[:, half * HD2:(half + 1) * HD2], ps_h2[:])
        if half == 1:
            # stage the already-transposed first half while the second
            # half's transposes run
            nc.gpsimd.dma_start(
                out=t["cc_mlp_in"][:, :DC * B // 2],
                in_=h2T[:, :DC // 2, :].rearrange("p c b -> p (c b)"))
    for c in range(DC // 2, DC):
        ps = psB.tile([P, 8], F16, tag="tp16", name=f"tp_h2_{c}")
        nc.tensor.transpose(ps[:, :B], h216[:, c * P:(c + 1) * P],
                            ident16[:B, :B])
        nc.vector.tensor_copy(out=h2T[:, c, :], in_=ps[:, :B])
    nc.gpsimd.dma_start(out=t["cc_mlp_in"][:, DC * B // 2:],
                        in_=h2T[:, DC // 2:, :].rearrange("p c b -> p (c b)"))
    nc.gpsimd.collective_compute(
        "AllReduce", ALU.add, replica_groups=GROUPS,
        ins=[t["cc_mlp_in"][:].opt()], outs=[t["cc_mlp_out"][:].opt()])
    h2Ts = singles.tile([P, DC, B], F16)
    nc.gpsimd.dma_start(out=h2Ts[:].rearrange("p c b -> p (c b)"),
                        in_=t["cc_mlp_out"][:])

    # =======================================================================
    # STEP 8: x0 += h2.T @ rin_pool (attn-part already accumulated)
    # =======================================================================
    for k in range(DC):
        nc.tensor.matmul(ps_x0[:], h2Ts[:, k, :], rp_sb[k // 16][:, k % 16, :],
                         start=False, stop=(k == DC - 1))
    x_nat = singles.tile([B, HID], F32)
    evict(x_nat[:], ps_x0[:])

    # ---- 3 residual blocks (mm1 weight-stationary -> hbT direct, no
    # hb transposes) ----
    for i in range(NBLK):
        xn16 = singles.tile([B, HID], F16, name=f"xn_{i}")
        layernorm16(x_nat[:], B, HID, xn16[:], f"lnb{i}")
        xnT = singles.tile([P, HC, B], F16, name=f"xnT_{i}")
        t_T16(xn16, xnT, HC, B, f"xn{i}",
              g=bgr_sb[:, i, :], b=bbr_sb[:, i, :])

        ps_h = psC.tile([P, 8 * B], F32, tag="mix", name=f"ps_bh_{i}")
        for m in range(8):
            for k in range(HC):
                nc.tensor.matmul(ps_h[:, m * B:(m + 1) * B],
                                 bw1_sb[i * HC + k][:, m * P:(m + 1) * P],
                                 xnT[:, k, :], start=(k == 0), stop=False)
            nc.tensor.matmul(ps_h[:, m * B:(m + 1) * B],
                             b1r_sb[:, i * 8 + m, :], ones8[:, :B],
                             start=False, stop=True)
        hbT = singles.tile([P, 8, B], F16, name=f"hbT_{i}")
        nc.scalar.activation(out=hbT[:].rearrange("p a b -> p (a b)"),
                             in_=ps_h[:], func=AF.Silu)

        ps_bo = psC.tile([B, HID], F32, tag="mix", name=f"ps_bo_{i}")
        for k in range(4 * HID // P):
            nc.tensor.matmul(ps_bo[:], hbT[:, k, :], bw2_sb[i][:, k, :],
                             start=(k == 0), stop=False)
        bias_mm(ps_bo, bb2_sb[:, i, :], HID)
        nc.vector.tensor_add(out=x_nat[:], in0=x_nat[:], in1=ps_bo[:])

    # ---- final: res (8, 7) = swish(x) @ out_w + out_b
    xs16 = singles.tile([B, HID], F16)
    nc.scalar.activation(out=xs16[:], in_=x_nat[:], func=AF.Silu)
    xsT = singles.tile([P, HC, B], F16)
    t_T16(xs16, xsT, HC, B, "xs")
    ps_o = psC.tile([B, AD], F32, tag="mix", name="ps_o")
    for k in range(HC):
        nc.tensor.matmul(ps_o[:], xsT[:, k, :], ow_sb[:, k, :],
                         start=(k == 0), stop=(k == HC - 1))
    out_sb = singles.tile([B, AD], F32)
    nc.vector.tensor_add(out=out_sb[:], in0=ps_o[:], in1=ob_bc[:])
    nc.sync.dma_start(out=t["res"][:], in_=out_sb[:])


_CACHED_NC = None


def _get_nc():
    global _CACHED_NC
    if _CACHED_NC is None:
        _CACHED_NC = build_program()
    return _CACHED_NC


def _prep_in_maps(inputs):
    f32 = np.float32
    f16 = np.float16
    f8 = ml_dtypes.float8_e3m4
    llm_full = np.ascontiguousarray(np.asarray(inputs["llm_output"], dtype=f32))
    wv = np.asarray(inputs["wv"], f32); wo = np.asarray(inputs["wo"], f32)
    bv = np.asarray(inputs["bv"], f32); bo = np.asarray(inputs["bo"], f32)
    w1 = np.asarray(inputs["mlp_w1"], f32); b1 = np.asarray(inputs["mlp_b1"], f32)
    w2 = np.asarray(inputs["mlp_w2"], f32); b2 = np.asarray(inputs["mlp_b2"], f32)
    rin_w = np.asarray(inputs["rin_w"], f32)

    # host-folded probe projection: U[:, h] = wk_h @ q_h / sqrt(DH)
    probe = np.asarray(inputs["probe"], np.float64).reshape(D)
    wq = np.asarray(inputs["wq"], np.float64)
    wk = np.asarray(inputs["wk"], np.float64)
    bq = np.asarray(inputs["bq"], np.float64)
    q = probe @ wq + bq
    U = np.zeros((D, H))
    for h in range(H):
        hb = slice(h * DH, (h + 1) * DH)
        U[:, h] = wk[:, hb] @ q[hb] / np.sqrt(DH)
    U16r = np.ascontiguousarray(
        (U / LLM_SCALE).reshape(DC, P, H).transpose(1, 0, 2)).astype(f16)

    def r128(v):  # (n*128,) -> (128, n) partition-major
        return np.ascontiguousarray(v.reshape(-1, P).T)

    blk_g = np.asarray(inputs["blk_ln_g"], f32)
    blk_b = np.asarray(inputs["blk_ln_b"], f32)
    blkw1 = np.asarray(inputs["blk_w1"], f32).astype(f16)    # (N, HID, 4HID)
    blkw2 = np.asarray(inputs["blk_w2"], f32).astype(f16)    # (N, 4HID, HID)
    rinp = np.ascontiguousarray(
        rin_w[TD:TD + D].reshape(DC, P, HID).transpose(1, 0, 2)).astype(f16)

    shared = {
        "U16r": U16r,
        "bo16": (bo / NC).astype(f16).reshape(1, D),
        "ln_g_r": r128(np.asarray(inputs["ln_g"], f32)),

        "b216": (b2 / NC).astype(f16).reshape(1, D),
        "four_w2": np.concatenate(
            [np.asarray(inputs["four_w"], f32).reshape(TD // 2, 1)] * 2),
        "phase2": np.concatenate(
            [np.full((TD // 2, 1), np.pi / 2, f32),
             np.zeros((TD // 2, 1), f32)]),
        "timeT": np.ascontiguousarray(np.asarray(inputs["time"], f32).T),
        "naT": np.ascontiguousarray(
            np.asarray(inputs["noisy_actions"], f32).T).astype(f16),
        "cond_w1": np.asarray(inputs["cond_w1"], f32).astype(f16),
        "cond_b1c": np.asarray(inputs["cond_b1"], f32).reshape(-1, 1),
        "cond_w2": np.asarray(inputs["cond_w2"], f32).astype(f16),
        "cond_b2c": np.asarray(inputs["cond_b2"], f32).reshape(-1, 1),
        "rin_cond": np.ascontiguousarray(rin_w[0:TD]).astype(f16),
        "rinp": rinp,
        "rin_na": np.ascontiguousarray(rin_w[TD + D:]).astype(f16),
        "rb16": np.asarray(inputs["rin_b"], f32).astype(f16).reshape(1, HID),
        "blk_g_r": np.ascontiguousarray(
            blk_g.reshape(NBLK, HC, P).transpose(0, 2, 1)),
        "blk_b_r": np.ascontiguousarray(
            blk_b.reshape(NBLK, HC, P).transpose(0, 2, 1)),
        "blkw1p": np.ascontiguousarray(
            blkw1.reshape(NBLK, HC, P, 4 * HID).transpose(2, 0, 1, 3)
            .reshape(P, NBLK * HC, 4 * HID)),
        "b1r": np.ascontiguousarray(
            np.asarray(inputs["blk_b1"], f32).astype(f16)
            .reshape(1, NBLK * 8, P)),
        "blkw2p": np.ascontiguousarray(
            blkw2.reshape(NBLK, 8, P, HID).transpose(2, 0, 1, 3)
            .reshape(P, NBLK, 8 * HID)),
        "blk_b2_16": np.asarray(inputs["blk_b2"], f32).astype(f16),
        "out_w": np.ascontiguousarray(
            np.asarray(inputs["out_w"], f32).astype(f16)
            .reshape(HC, P, AD).transpose(1, 0, 2)),
        "out_bc": np.asarray(inputs["out_b"], f32).reshape(1, AD),
    }

    # host-folded LayerNorm rows for mm1: r1 = g@w1, br2 = lnb@w1 + b1
    w1_64 = w1.astype(np.float64)
    r1_full = np.asarray(inputs["ln_g"], np.float64) @ w1_64
    br2_full = (np.asarray(inputs["ln_b"], np.float64) @ w1_64
                + b1.astype(np.float64))

    in_maps = []
    for i in range(NC):
        hb = slice(i * DH, (i + 1) * DH)
        fb = slice(i * F1S, (i + 1) * F1S)
        m = dict(shared)
        m["llm16"] = llm_full[i].astype(f16)
        m["llmT8"] = np.clip(
            llm_full[i].T * LLM_SCALE, -15.5, 15.5).astype(f8)
        m["wv_s"] = np.ascontiguousarray(wv[:, hb]).astype(f16)
        m["bv16"] = np.ascontiguousarray(bv[hb]).astype(f16).reshape(1, DH)
        m["wo_s"] = np.ascontiguousarray(wo[hb, :]).astype(f16)
        m["w1_s"] = np.ascontiguousarray(w1[:, fb]).astype(f16)
        m["r1c"] = r1_full[fb].astype(f16).reshape(1, F1S)
        m["br2"] = br2_full[fb].astype(f16).reshape(1, F1S)
        m["w2_s"] = np.ascontiguousarray(w2[fb, :]).astype(f16)
        in_maps.append(m)
    return in_maps


def kernel(**inputs):
    nc = _get_nc()
    in_maps = _prep_in_maps(inputs)
    r = run_bass_kernel_spmd(nc, in_maps, core_ids=list(range(NC)))
    return np.ascontiguousarray(r.results[0]["res"]).astype(np.float32)


def run_traced(**inputs):
    """Like kernel() but with NTFF tracing; returns (output, results)."""
    nc = _get_nc()
    in_maps = _prep_in_maps(inputs)
    r = run_bass_kernel_spmd(nc, in_maps, core_ids=list(range(NC)), trace=True)
    return np.ascontiguousarray(r.results[0]["res"]).astype(np.float32), r
